# revision 1
# baseline (speedup 1.0000x reference)
"""Trainium2 Bass kernel for nn_KGVAE (2-layer BDD RelGraphConv + gaussian sample).

Strategy (8 NeuronCores, SPMD):
  - Nodes are range-sharded by id across the 8 cores (12500 each, padded to
    12544 = 98 blocks of 128).  Each core owns all edges whose dst lands in
    its node range, so per-layer aggregates need no cross-core reduction.
  - Within a core, nodes are re-bucketed into 128-node blocks by LPT on
    in-degree so every block sees a near-equal number of incoming edges.
  - Layer compute is two phases over a DRAM message buffer laid out in
    dst-block order:
      Phase A (edges grouped by relation): indirect-DMA gather h[src],
        PE-transpose, matmul with the relation's expanded block-diagonal
        weight, scale by per-edge norm, indirect-DMA scatter the message row
        to its dst-sorted position.
      Phase B (per 128-node block): stream message tiles, build a one-hot
        slot matrix on-device (iota + is_equal against dst%128), and
        accumulate P^T @ M into PSUM together with the self-loop matmul;
        epilogue adds bias and applies the activation.
  - The two conv layers run as two SPMD dispatches; the host concatenates
    the per-core layer-1 outputs into the (permuted) full feature matrix
    that layer 2 gathers from.  The gaussian head is fused into dispatch 2.

Self-contained: hardcodes nothing about file layout; all shapes derive from
the passed-in arrays.
"""

import heapq
import sys
import time
from contextlib import ExitStack

import numpy as np

import concourse.bass as bass
import concourse.mybir as mybir
import concourse.tile as tile
from concourse import bacc
from concourse.bass_utils import run_bass_kernel_spmd

P = 128
NCORES = 8
TRACE = False  # set True by a harness to collect exec times
LAST_EXEC_NS = []

F32 = mybir.dt.float32
F32R = mybir.dt.float32r
I32 = mybir.dt.int32
USE_F32R = True  # tf32-style matmuls where the moving dim is >=256
AF = mybir.ActivationFunctionType
ALU = mybir.AluOpType


def _cdiv(a, b):
    return -(-a // b)


def _expand_bd(W):
    """(R, NB, si, so) block weights -> (R, NB*si, NB*so) dense block-diagonal."""
    R, NB, si, so = W.shape
    out = np.zeros((R, NB * si, NB * so), dtype=np.float32)
    for b in range(NB):
        out[:, b * si:(b + 1) * si, b * so:(b + 1) * so] = W[:, b]
    return out


def _rank_within_group(sorted_keys):
    """For a sorted key array, return rank of each element within its run."""
    n = len(sorted_keys)
    if n == 0:
        return np.zeros(0, dtype=np.int64)
    starts = np.searchsorted(sorted_keys, sorted_keys, side="left")
    return np.arange(n, dtype=np.int64) - starts


def _plan(src, dst, etype, norm, N, R):
    """Host-side edge partitioning/scheduling. Returns dict of constants and
    per-core arrays shared by both dispatches."""
    E = len(src)
    NPC = N // NCORES
    assert NPC * NCORES == N
    NPAD = _cdiv(NPC, P) * P
    NBLK = NPAD // P

    deg = np.bincount(dst, minlength=N)

    # --- LPT assignment of each core's nodes into 128-node blocks ---
    block_of = np.empty(N, np.int32)
    slot_of = np.empty(N, np.int32)
    orig_of_slot = np.full((NCORES, NPAD), -1, np.int64)  # pnode -> original id
    block_loads = np.zeros((NCORES, NBLK), np.int64)
    for c in range(NCORES):
        ids = np.arange(c * NPC, (c + 1) * NPC, dtype=np.int64)
        d = deg[ids]
        order = np.argsort(-d, kind="stable")
        heap = [(0, q) for q in range(NBLK)]
        heapq.heapify(heap)
        counts = np.zeros(NBLK, np.int32)
        for i in order:
            while True:
                load, q = heapq.heappop(heap)
                if counts[q] < P:
                    break
            node = ids[i]
            block_of[node] = q
            slot_of[node] = counts[q]
            orig_of_slot[c, q * P + counts[q]] = node
            counts[q] += 1
            heapq.heappush(heap, (load + int(d[i]), q))
        np.add.at(block_loads[c], block_of[ids], d)

    # permuted global index of each node (for layer-2 gathers)
    pg = np.empty(N, np.int64)
    for c in range(NCORES):
        ids = np.arange(c * NPC, (c + 1) * NPC, dtype=np.int64)
        pg[ids] = c * NPAD + block_of[ids].astype(np.int64) * P + slot_of[ids]

    T_B = max(1, _cdiv(int(block_loads.max()), P))
    B_slots = NBLK * T_B * P
    B_tiles = NBLK * T_B

    core_of_edge = dst // NPC

    # --- per-relation tile counts, shared across cores (SPMD) ---
    cnt = np.zeros((NCORES, R), np.int64)
    for c in range(NCORES):
        cnt[c] = np.bincount(etype[core_of_edge == c], minlength=R)
    T1 = _cdiv(cnt.max(axis=0), P)  # tiles per relation
    A_tiles = int(T1.sum())
    if A_tiles * P < B_slots:
        T1[int(np.argmax(T1))] += _cdiv(B_slots - A_tiles * P, P)
        A_tiles = int(T1.sum())
    A_slots = A_tiles * P
    rel_tile_off = np.concatenate([[0], np.cumsum(T1)])  # tile offset per rel

    TRASH = B_slots  # scatter target for surplus pad slots

    per_core = []
    for c in range(NCORES):
        eidx = np.nonzero(core_of_edge == c)[0]
        Ec = len(eidx)
        e_src = src[eidx]
        e_dst = dst[eidx]
        e_rel = etype[eidx]
        e_norm = norm[eidx].reshape(-1).astype(np.float32)

        q_e = block_of[e_dst].astype(np.int64)
        p_e = slot_of[e_dst].astype(np.int64)

        # msg-buffer row for each edge: block-major, sequential within block
        order_q = np.argsort(q_e, kind="stable")
        jq = _rank_within_group(q_e[order_q])
        row = np.empty(Ec, np.int64)
        row[order_q] = q_e[order_q] * (T_B * P) + jq

        dstp = np.full(B_slots, 200.0, np.float32)
        dstp[row] = p_e.astype(np.float32)

        # phase-A slot for each edge: relation-major regions
        order_r = np.argsort(e_rel, kind="stable")
        jr = _rank_within_group(e_rel[order_r])
        slotA = np.empty(Ec, np.int64)
        slotA[order_r] = rel_tile_off[e_rel[order_r]] * P + jr

        srcA1 = np.zeros(A_slots, np.int32)
        srcA2 = np.zeros(A_slots, np.int32)
        normA = np.zeros(A_slots, np.float32)
        posA = np.full(A_slots, TRASH, np.int32)
        srcA1[slotA] = e_src.astype(np.int32)
        srcA2[slotA] = pg[e_src].astype(np.int32)
        normA[slotA] = e_norm
        posA[slotA] = row.astype(np.int32)

        # route pad slots to the unfilled message rows (so they read as 0.0)
        used = np.zeros(A_slots, bool)
        used[slotA] = True
        apad = np.nonzero(~used)[0]
        brow_used = np.zeros(B_slots, bool)
        brow_used[row] = True
        bpad = np.nonzero(~brow_used)[0]
        assert len(apad) >= len(bpad), (len(apad), len(bpad))
        posA[apad[: len(bpad)]] = bpad.astype(np.int32)

        per_core.append(
            dict(
                srcA1_t=np.ascontiguousarray(srcA1.reshape(A_tiles, P).T),
                srcA2_t=np.ascontiguousarray(srcA2.reshape(A_tiles, P).T),
                normA_t=np.ascontiguousarray(normA.reshape(A_tiles, P).T),
                posA_t=np.ascontiguousarray(posA.reshape(A_tiles, P).T),
                dstp_t=np.ascontiguousarray(dstp.reshape(B_tiles, P).T),
            )
        )

    return dict(
        NPC=NPC, NPAD=NPAD, NBLK=NBLK, T_B=T_B, T1=T1,
        A_tiles=A_tiles, B_tiles=B_tiles, B_slots=B_slots,
        orig_of_slot=orig_of_slot, per_core=per_core, R=R,
    )


def _build_layer(plan, F, n_src_rows, gauss):
    """Build one SPMD dispatch program. F = per-edge message width.
    n_src_rows = rows of the gather-source feature matrix.
    gauss=False -> relu + write h [NPAD, F];  gauss=True -> gaussian head,
    write z [NPAD, 128]."""
    NPAD, NBLK, T_B = plan["NPAD"], plan["NBLK"], plan["T_B"]
    A_tiles, B_tiles = plan["A_tiles"], plan["B_tiles"]
    B_slots = plan["B_slots"]
    T1, R = plan["T1"], plan["R"]
    H = 128

    nc = bacc.Bacc("TRN2", target_bir_lowering=False, debug=False)

    hsrc = nc.dram_tensor("hsrc", [n_src_rows, H], F32, kind="ExternalInput")
    Wd = nc.dram_tensor("W", [R, H, F], F32, kind="ExternalInput")
    loopw = nc.dram_tensor("loopw", [H, F], F32, kind="ExternalInput")
    biasb = nc.dram_tensor("biasb", [P, F], F32, kind="ExternalInput")
    srcA_d = nc.dram_tensor("srcA", [P, A_tiles], I32, kind="ExternalInput")
    normA_d = nc.dram_tensor("normA", [P, A_tiles], F32, kind="ExternalInput")
    posA_d = nc.dram_tensor("posA", [P, A_tiles], I32, kind="ExternalInput")
    dstp_d = nc.dram_tensor("dstp", [P, B_tiles], F32, kind="ExternalInput")
    hloc = nc.dram_tensor("hloc", [NPAD, H], F32, kind="ExternalInput")
    iota_d = nc.dram_tensor("iota", [P, P], F32, kind="ExternalInput")
    if gauss:
        epsl = nc.dram_tensor("epsl", [NPAD, H], F32, kind="ExternalInput")
        out_d = nc.dram_tensor("out", [NPAD, H], F32, kind="ExternalOutput")
    else:
        out_d = nc.dram_tensor("out", [NPAD, F], F32, kind="ExternalOutput")
    # message buffer in dst-block order (one extra tile holds trash writes)
    msgbuf = nc.dram_tensor("msgbuf", [B_slots + P, F], F32)

    with tile.TileContext(nc) as tc, ExitStack() as ctx:
        const = ctx.enter_context(tc.tile_pool(name="const", bufs=1))
        apool = ctx.enter_context(tc.tile_pool(name="apool", bufs=6))
        wpool = ctx.enter_context(tc.tile_pool(name="wpool", bufs=3))
        bpool = ctx.enter_context(tc.tile_pool(name="bpool", bufs=4))
        papool = ctx.enter_context(tc.tile_pool(name="papool", bufs=3, space="PSUM"))
        pbpool = ctx.enter_context(tc.tile_pool(name="pbpool", bufs=2, space="PSUM"))

        # resident constants / metadata
        ident = const.tile([P, P], F32)
        iota_sb = const.tile([P, P], F32)
        loopw_sb = const.tile([H, F], F32)
        bias_sb = const.tile([P, F], F32)
        srcA_sb = const.tile([P, A_tiles], I32)
        normA_sb = const.tile([P, A_tiles], F32)
        posA_sb = const.tile([P, A_tiles], I32)
        dstp_sb = const.tile([P, B_tiles], F32)
        nc.sync.dma_start(out=iota_sb[:], in_=iota_d[:])
        nc.sync.dma_start(out=loopw_sb[:], in_=loopw[:])
        nc.sync.dma_start(out=bias_sb[:], in_=biasb[:])
        nc.sync.dma_start(out=srcA_sb[:], in_=srcA_d[:])
        nc.sync.dma_start(out=normA_sb[:], in_=normA_d[:])
        nc.sync.dma_start(out=posA_sb[:], in_=posA_d[:])
        nc.sync.dma_start(out=dstp_sb[:], in_=dstp_d[:])
        from concourse.masks import make_identity
        make_identity(nc, ident[:])
        if gauss:
            eps_bias = const.tile([P, 1], F32)
            nc.vector.memset(eps_bias[:], 1e-8)
            one_bias = const.tile([P, 1], F32)
            nc.vector.memset(one_bias[:], 1.0)

        # ---------------- phase A: per-edge messages ----------------
        k = 0
        for r in range(R):
            nt = int(T1[r])
            if nt == 0:
                continue
            w_sb = wpool.tile([H, F], F32, tag="w")
            nc.sync.dma_start(out=w_sb[:], in_=Wd[r])
            for _t in range(nt):
                h_t = apool.tile([P, H], F32, tag="h")
                nc.gpsimd.indirect_dma_start(
                    out=h_t[:], out_offset=None, in_=hsrc[:],
                    in_offset=bass.IndirectOffsetOnAxis(ap=srcA_sb[:, k:k + 1], axis=0),
                )
                hT_ps = papool.tile([P, P], F32, tag="hT_ps")
                nc.tensor.transpose(out=hT_ps[:], in_=h_t[:], identity=ident[:])
                hT_sb = apool.tile([P, P], F32, tag="hT")
                nc.scalar.activation(out=hT_sb[:], in_=hT_ps[:], func=AF.Copy)
                msg_ps = papool.tile([P, F], F32, tag="msg_ps")
                nc.tensor.matmul(out=msg_ps[:], lhsT=hT_sb[:], rhs=w_sb[:],
                                 start=True, stop=True)
                msg_sb = apool.tile([P, F], F32, tag="msg")
                nc.vector.tensor_scalar(
                    out=msg_sb[:], in0=msg_ps[:],
                    scalar1=normA_sb[:, k:k + 1], scalar2=None, op0=ALU.mult,
                )
                nc.gpsimd.indirect_dma_start(
                    out=msgbuf[:],
                    out_offset=bass.IndirectOffsetOnAxis(ap=posA_sb[:, k:k + 1], axis=0),
                    in_=msg_sb[:], in_offset=None,
                )
                k += 1
        assert k == A_tiles

        tc.strict_bb_all_engine_barrier()

        # ---------------- phase B: per-block aggregation ----------------
        for q in range(NBLK):
            out_ps = pbpool.tile([P, F], F32, tag="out_ps")
            hl_t = bpool.tile([P, H], F32, tag="hl")
            nc.sync.dma_start(out=hl_t[:], in_=hloc[q * P:(q + 1) * P, :])
            hlT_ps = papool.tile([P, P], F32, tag="hT_ps")
            nc.tensor.transpose(out=hlT_ps[:], in_=hl_t[:], identity=ident[:])
            hlT_sb = apool.tile([P, P], F32, tag="hT")
            nc.scalar.activation(out=hlT_sb[:], in_=hlT_ps[:], func=AF.Copy)
            nc.tensor.matmul(out=out_ps[:], lhsT=hlT_sb[:], rhs=loopw_sb[:],
                             start=True, stop=False)

            mblk = bpool.tile([P, T_B * F], F32, tag="mblk")
            blk_view = msgbuf[q * T_B * P:(q + 1) * T_B * P, :].rearrange(
                "(t p) f -> p t f", p=P)
            nc.sync.dma_start(
                out=mblk[:].rearrange("p (t f) -> p t f", f=F), in_=blk_view)
            for t in range(T_B):
                kb = q * T_B + t
                P_t = bpool.tile([P, P], F32, tag="Pt")
                nc.vector.tensor_scalar(
                    out=P_t[:], in0=iota_sb[:],
                    scalar1=dstp_sb[:, kb:kb + 1], scalar2=None, op0=ALU.is_equal,
                )
                nc.tensor.matmul(out=out_ps[:], lhsT=P_t[:],
                                 rhs=mblk[:, t * F:(t + 1) * F],
                                 start=False, stop=(t == T_B - 1))

            hb = bpool.tile([P, F], F32, tag="hb")
            nc.vector.tensor_tensor(out=hb[:], in0=out_ps[:], in1=bias_sb[:],
                                    op=ALU.add)
            if not gauss:
                nc.scalar.activation(out=hb[:], in_=hb[:], func=AF.Relu)
                nc.sync.dma_start(out=out_d[q * P:(q + 1) * P, :], in_=hb[:])
            else:
                # softplus(x) = relu(x) + ln(1 + exp(-|x|)); sqrt(v) = exp(ln(v)/2)
                # (gen3 has no softplus/sqrt table; Exp+Ln live in one table)
                sq = bpool.tile([P, H], F32, tag="sq")
                ax = bpool.tile([P, H], F32, tag="ax")
                nc.scalar.activation(out=ax[:], in_=hb[:, H:2 * H], func=AF.Abs)
                nc.scalar.activation(out=ax[:], in_=ax[:], func=AF.Exp, scale=-1.0)
                nc.scalar.activation(out=ax[:], in_=ax[:], func=AF.Ln,
                                     bias=one_bias[:])
                nc.scalar.activation(out=sq[:], in_=hb[:, H:2 * H], func=AF.Relu)
                nc.vector.tensor_tensor(out=sq[:], in0=sq[:], in1=ax[:], op=ALU.add)
                nc.scalar.activation(out=sq[:], in_=sq[:], func=AF.Ln,
                                     bias=eps_bias[:])
                nc.scalar.activation(out=sq[:], in_=sq[:], func=AF.Exp, scale=0.5)
                ep = bpool.tile([P, H], F32, tag="ep")
                nc.sync.dma_start(out=ep[:], in_=epsl[q * P:(q + 1) * P, :])
                z_t = bpool.tile([P, H], F32, tag="z")
                nc.vector.tensor_tensor(out=z_t[:], in0=sq[:], in1=ep[:], op=ALU.mult)
                nc.vector.tensor_tensor(out=z_t[:], in0=z_t[:], in1=hb[:, :H],
                                        op=ALU.add)
                nc.sync.dma_start(out=out_d[q * P:(q + 1) * P, :], in_=z_t[:])

    nc.compile()
    return nc


def _log(msg):
    print(f"[kernel] {msg}", file=sys.stderr, flush=True)


def _run(nc, in_maps, label):
    global LAST_EXEC_NS
    t0 = time.time()
    try:
        res = run_bass_kernel_spmd(
            nc, in_maps, core_ids=list(range(NCORES)), trace=TRACE,
        )
    except (ImportError, ModuleNotFoundError):
        res = run_bass_kernel_spmd(
            nc, in_maps, core_ids=list(range(NCORES)), trace=False,
        )
    _log(f"{label} run {time.time() - t0:.1f}s exec_ns={res.exec_time_ns}")
    if TRACE:
        # compiled executable is warm now; second call approximates
        # dispatch+exec wall time (includes input transfer)
        t1 = time.time()
        run_bass_kernel_spmd(nc, in_maps, core_ids=list(range(NCORES)),
                             trace=False)
        wall_ns = int((time.time() - t1) * 1e9)
        _log(f"{label} rerun wall {wall_ns / 1e6:.1f} ms")
        LAST_EXEC_NS.append((label, res.exec_time_ns, wall_ns))
    return res.results


def kernel(node_ids, src, dst, etype, norm, emb, W1, loop1, b1, W2, loop2, b2, eps):
    node_ids = np.asarray(node_ids).astype(np.int64)
    src = np.asarray(src).astype(np.int64)
    dst = np.asarray(dst).astype(np.int64)
    etype = np.asarray(etype).astype(np.int64)
    norm = np.asarray(norm, np.float32)
    emb = np.asarray(emb, np.float32)
    W1 = np.asarray(W1, np.float32)
    loop1 = np.asarray(loop1, np.float32)
    b1 = np.asarray(b1, np.float32)
    W2 = np.asarray(W2, np.float32)
    loop2 = np.asarray(loop2, np.float32)
    b2 = np.asarray(b2, np.float32)
    eps = np.asarray(eps, np.float32)

    N, H = emb.shape
    R = W1.shape[0]
    F2 = W2.shape[1] * W2.shape[3]
    assert H == 128

    h0 = emb[node_ids]
    t0 = time.time()
    plan = _plan(src, dst, etype, norm, N, R)
    _log(f"plan {time.time() - t0:.1f}s A_tiles={plan['A_tiles']} "
         f"B_tiles={plan['B_tiles']} T_B={plan['T_B']}")
    NPAD, NPC = plan["NPAD"], plan["NPC"]
    orig_of_slot = plan["orig_of_slot"]

    W1bd = _expand_bd(W1)  # (R, 128, 128)
    W2bd = _expand_bd(W2)  # (R, 128, 256)
    iota = np.tile(np.arange(P, dtype=np.float32), (P, 1))
    bias1b = np.tile(b1.astype(np.float32), (P, 1))
    bias2b = np.tile(b2.astype(np.float32), (P, 1))

    # per-core permuted local feature/eps rows (dummy slots -> 0)
    hloc_c, eps_c = [], []
    for c in range(NCORES):
        sl = orig_of_slot[c]
        valid = sl >= 0
        hl = np.zeros((NPAD, H), np.float32)
        hl[valid] = h0[sl[valid]]
        ev = np.zeros((NPAD, H), np.float32)
        ev[valid] = eps[sl[valid]]
        hloc_c.append(hl)
        eps_c.append(ev)

    # ---- dispatch 1 ----
    t0 = time.time()
    nc1 = _build_layer(plan, F=H, n_src_rows=N, gauss=False)
    _log(f"build1 {time.time() - t0:.1f}s")
    in_maps1 = []
    for c in range(NCORES):
        pc = plan["per_core"][c]
        in_maps1.append(dict(
            hsrc=h0, W=W1bd, loopw=loop1.astype(np.float32),
            biasb=bias1b, srcA=pc["srcA1_t"], normA=pc["normA_t"],
            posA=pc["posA_t"], dstp=pc["dstp_t"], hloc=hloc_c[c], iota=iota,
        ))
    res1 = _run(nc1, in_maps1, "layer1")
    h1full = np.concatenate([res1[c]["out"] for c in range(NCORES)], axis=0)

    # ---- dispatch 2 ----
    t0 = time.time()
    nc2 = _build_layer(plan, F=F2, n_src_rows=NCORES * NPAD, gauss=True)
    _log(f"build2 {time.time() - t0:.1f}s")
    in_maps2 = []
    for c in range(NCORES):
        pc = plan["per_core"][c]
        in_maps2.append(dict(
            hsrc=h1full, W=W2bd, loopw=loop2.astype(np.float32),
            biasb=bias2b, srcA=pc["srcA2_t"], normA=pc["normA_t"],
            posA=pc["posA_t"], dstp=pc["dstp_t"],
            hloc=h1full[c * NPAD:(c + 1) * NPAD], iota=iota, epsl=eps_c[c],
        ))
    res2 = _run(nc2, in_maps2, "layer2")

    z = np.empty((N, H), np.float32)
    for c in range(NCORES):
        sl = orig_of_slot[c]
        valid = sl >= 0
        z[sl[valid]] = res2[c]["out"][valid]
    return z

